# revision 1
# baseline (speedup 1.0000x reference)
"""GCN + DiffPool kernel for Trainium2, data-parallel over graphs across 8 NeuronCores.

Model (per graph, n=150 nodes):
  Z1 = relu(An @ (x @ W1) + b1)          An = D^-1/2 (A+I) D^-1/2
  Z2 = relu(An @ (Z1 @ W2) + b2)
  S  = softmax(An @ (Z2 @ Wa) + ba)      [n, 25]
  Zp = S^T @ Z2 ; Ap = S^T @ (A @ S)
  H  = relu(Anp @ (Zp @ Wp) + bp)        pooled GCN, 25 cluster-nodes
  logits = (sum_rows H) @ Wc + bc

Sharding: 64 graphs -> 8 devices x 8 graphs. The batch adjacency is block
diagonal, so each device only receives its 8 graphs' 150x150 diagonal blocks
(packed into a [128,8,150] + [22,8,150] partition-chunk layout) and its node
rows of x (shipped feature-major). Everything is graph-local; the final [8,10]
logits per device are concatenated on host.

On-device layout convention:
  fm (feature-major): [feat_part, graph, node]  - used for W-multiplies (lhsT)
  nm (node-major):    [node_part, graph, feat]  - used for A-multiplies
A-multiplies contract over nodes, so node dim (150) is split into partition
chunks c0=[0:128], c1=[128:150]. Normalization is folded: the row factor
D^-1/2 is applied to the moving operand; the column factor is materialized
once as An_col = (A+I) * dT_broadcast.
"""

import numpy as np

import concourse.bass as bass
import concourse.mybir as mybir
import concourse.tile as tile
from concourse import bacc
from concourse.bass_utils import run_bass_kernel_spmd

F32 = mybir.dt.float32
BF16 = mybir.dt.bfloat16
AF = mybir.ActivationFunctionType

# matmul-operand dtype (accumulation is always fp32 in PSUM)
MMDT = BF16

N_NODES = 9600
N_FEAT = 128
HIDDEN = 64
CLUSTERS = 25
NUM_CLASSES = 10
B_GRAPHS = 64
NPG = 150            # nodes per graph
DEV = 8              # devices
GPD = 8              # graphs per device
C0, C1 = 128, 22     # node partition chunks (128 + 22 = 150)

_CACHE = {}


def _chunk(c):
    """(offset, size) of node chunk c."""
    return (0, C0) if c == 0 else (C0, C1)
# packed-constant column offsets
WP_W1, WP_W2, WP_WA, WP_WP, WP_BA, WP_ID128, WP_ONES = 0, 64, 128, 153, 217, 242, 370
WP_ONEC = 520
WP_COLS = 521
FP_WC, FP_B1, FP_B2, FP_BP, FP_BC, FP_ID25, FP_ONE = 0, 10, 11, 12, 13, 23, 48
FP_COLS = 49


def build_nc():
    nc = bacc.Bacc("TRN2", target_bir_lowering=False, debug=False, num_devices=DEV)

    def din(name, shape, dt=F32):
        return nc.dram_tensor(name, shape, dt, kind="ExternalInput").ap()

    xT = din("xT", [N_FEAT, GPD, NPG], MMDT)
    a0 = din("a0", [C0, GPD, NPG], MMDT)
    a1 = din("a1", [C1, GPD, NPG], MMDT)
    idp = din("idp", [C0, 2 * NPG], MMDT)
    wpk = din("wpk", [N_FEAT, WP_COLS], MMDT)
    fpk = din("fpk", [N_FEAT, FP_COLS], F32)
    out = nc.dram_tensor("out", [GPD, NUM_CLASSES], F32, kind="ExternalOutput").ap()

    with tile.TileContext(nc) as tc:
        with (
            tc.tile_pool(name="cst", bufs=1) as cst,
            tc.tile_pool(name="act", bufs=1) as act,
            tc.tile_pool(name="ps", bufs=7, space="PSUM") as ps,
            tc.tile_pool(name="pst", bufs=1, space="PSUM") as pst,
            tc.tile_pool(name="dram", bufs=1, space="DRAM") as dram,
        ):
            # ---- load inputs (HW-DGE on sync engine; a0 first: it heads the
            # critical degree->dT->An chain) --------------------------------
            H2 = GPD // 2
            s_a0 = cst.tile([C0, GPD, NPG], MMDT, tag="a0")
            HG = GPD // 2
            nc.sync.dma_start(out=s_a0[:, 0:HG, :], in_=a0[:, 0:HG, :])
            nc.scalar.dma_start(out=s_a0[:, HG:GPD, :], in_=a0[:, HG:GPD, :])
            s_a1 = cst.tile([C1, GPD, NPG], MMDT, tag="a1")
            nc.sync.dma_start(out=s_a1[:], in_=a1)
            s_xT = cst.tile([N_FEAT, GPD, NPG], MMDT, tag="xT")
            nc.gpsimd.dma_start(out=s_xT[:], in_=xT)
            s_idp = cst.tile([C0, 2 * NPG], MMDT, tag="idp")
            nc.sync.dma_start(out=s_idp[:], in_=idp)
            s_wpk = cst.tile([N_FEAT, WP_COLS], MMDT, tag="wpk")
            nc.gpsimd.dma_start(out=s_wpk[:], in_=wpk)
            s_fpk = cst.tile([N_FEAT, FP_COLS], F32, tag="fpk")
            nc.gpsimd.dma_start(out=s_fpk[:], in_=fpk)

            s_a = (s_a0, s_a1)
            s_id = (s_idp[:, 0:NPG], s_idp[0:C1, NPG:2 * NPG])
            s_W1 = s_wpk[:, WP_W1:WP_W1 + HIDDEN]
            s_W2 = s_wpk[0:HIDDEN, WP_W2:WP_W2 + HIDDEN]
            s_Wa = s_wpk[0:HIDDEN, WP_WA:WP_WA + CLUSTERS]
            s_Wp = s_wpk[0:HIDDEN, WP_WP:WP_WP + HIDDEN]
            s_baR = s_wpk[0:1, WP_BA:WP_BA + CLUSTERS]
            s_id128 = s_wpk[:, WP_ID128:WP_ID128 + N_FEAT]
            s_id64 = s_id128[0:HIDDEN, 0:HIDDEN]
            s_Wc = s_fpk[0:HIDDEN, FP_WC:FP_WC + NUM_CLASSES]
            s_b1 = s_fpk[0:HIDDEN, FP_B1:FP_B1 + 1]
            s_b2 = s_fpk[0:HIDDEN, FP_B2:FP_B2 + 1]
            s_bp = s_fpk[0:HIDDEN, FP_BP:FP_BP + 1]
            s_bc = s_fpk[0:GPD, FP_BC:FP_BC + NUM_CLASSES]

            # ---- degrees + d = rsqrt(deg+1), all on DVE -------------------
            # Quake bit-trick seed + 2 Newton steps (~5e-6 rel err), no ACT
            # tables involved (keeps the single Exp/Relu table resident).
            U32 = mybir.dt.uint32
            AL = mybir.AluOpType
            qk1 = act.tile([C0, 1], U32, tag="qk1")
            nc.vector.memset(qk1[:], 1)
            qkm = act.tile([C0, 1], U32, tag="qkm")
            nc.vector.memset(qkm[:], 0x5F3759DF)

            def emit_rsqrt(x, rows, cols, iters=2):
                """x (tile [rows, cols], holds arg) -> rsqrt(x) tile, fp32."""
                s = act.tile([rows, cols], F32, tag=f"rs_{x.name if hasattr(x,'name') else id(x)}")
                w = act.tile([rows, cols], F32, tag=f"rw_{id(x)}")
                nc.vector.tensor_tensor(s[:].bitcast(U32), x[:].bitcast(U32),
                                        qk1[0:rows, :].broadcast_to((rows, cols)),
                                        AL.logical_shift_right)
                nc.vector.tensor_tensor(s[:].bitcast(U32),
                                        qkm[0:rows, :].broadcast_to((rows, cols)),
                                        s[:].bitcast(U32), AL.subtract)
                for _ in range(iters):
                    nc.vector.tensor_mul(w[:], s[:], s[:])
                    nc.vector.tensor_mul(w[:], w[:], x[:])
                    nc.vector.tensor_scalar(w[:], w[:], -0.5, 1.5, AL.mult, AL.add)
                    nc.vector.tensor_mul(s[:], s[:], w[:])
                return s

            degc = act.tile([C0, 2 * GPD], F32, tag="degc")
            nc.vector.memset(degc[0:C0, GPD:2 * GPD], 1.0)  # keep unused region finite
            nc.vector.reduce_sum(out=degc[:, 0:HG], in_=s_a0[:, 0:HG, :],
                                 axis=mybir.AxisListType.X)
            nc.vector.reduce_sum(out=degc[:, HG:GPD], in_=s_a0[:, HG:GPD, :],
                                 axis=mybir.AxisListType.X)
            nc.vector.reduce_sum(out=degc[0:C1, GPD:2 * GPD], in_=s_a1[:],
                                 axis=mybir.AxisListType.X)
            nc.vector.tensor_scalar_add(degc[:], degc[:], 1.0)
            dcomb = emit_rsqrt(degc, C0, 2 * GPD)
            s_d = [dcomb[:, 0:GPD], dcomb[0:C1, GPD:2 * GPD]]
            dbfc = act.tile([C0, 2 * GPD], MMDT, tag="dbfc")
            nc.vector.tensor_copy(dbfc[:], dcomb[:])
            s_dbf = [dbfc[:, 0:GPD], dbfc[0:C1, GPD:2 * GPD]]

            # ---- dT broadcast via DRAM roundtrip ----------------------------
            # d is [node_chunk, graph]; we need dT_bc[p, g, j] = d_g[j] on all
            # partitions. Scatter to DRAM in (g, j) layout, read back with a
            # partition-broadcast AP.
            p_dt = pst.tile([GPD, C0], MMDT, tag="p2")
            nc.tensor.transpose(p_dt[:, 0:C0], s_dbf[0][:], s_id128)
            p_dt2 = pst.tile([GPD, C1], MMDT, tag="p2")
            nc.tensor.transpose(p_dt2[:, 0:C1], s_dbf[1][:], s_id128[0:C1, 0:C1])
            dTrow = act.tile([GPD, NPG], MMDT, tag="dTrow")
            nc.vector.tensor_copy(dTrow[:, 0:C0], p_dt[:])
            nc.vector.tensor_copy(dTrow[:, C0:NPG], p_dt2[:])
            dTd = dram.tile([GPD * NPG], MMDT, tag="dTd")
            nc.sync.dma_start(out=dTd[:].rearrange("(g j) -> g j", g=GPD), in_=dTrow[:])
            s_dT = cst.tile([C0, GPD, NPG], MMDT, tag="dT")
            dT_src = dTd[:].rearrange("(g j) -> g j", g=GPD)[None, :, :] \
                .broadcast_to((C0, GPD, NPG))
            Q = GPD // 4
            for q in range(4):
                eng = nc.sync if q % 2 == 0 else nc.scalar
                eng.dma_start(out=s_dT[:, q * Q:(q + 1) * Q, :],
                              in_=dT_src[:, q * Q:(q + 1) * Q, :])

            # ---- An_col = (A + I) * dT_bc -----------------------------------
            # chunk-1 tile has an extra all-ones contraction row (partition 22)
            # used to fold the +ba bias into the assignment matmul.
            s_An = []
            for c, cn in ((0, C0), (1, C1)):
                ah = act.tile([cn, GPD, NPG], MMDT, tag=f"ah{c}")
                idb = s_id[c][:, None, :].broadcast_to((cn, GPD, NPG))
                nc.vector.tensor_add(ah[:], s_a[c][:], idb)
                an = act.tile([cn + (1 if c == 1 else 0), GPD, NPG], MMDT, tag=f"an{c}")
                if c == 0:
                    for q in range(4):
                        nc.vector.tensor_mul(an[0:cn, q * Q:(q + 1) * Q, :],
                                             ah[:, q * Q:(q + 1) * Q, :],
                                             s_dT[0:cn, q * Q:(q + 1) * Q, :])
                else:
                    nc.vector.tensor_mul(an[0:cn, 0:H2, :], ah[:, 0:H2, :],
                                         s_dT[0:cn, 0:H2, :])
                    nc.vector.tensor_mul(an[0:cn, H2:GPD, :], ah[:, H2:GPD, :],
                                         s_dT[0:cn, H2:GPD, :])
                s_An.append(an)
            ones_src = wpk[0:1, WP_ONES:WP_ONES + NPG][:, None, :] \
                .broadcast_to((1, GPD, NPG))
            nc.gpsimd.dma_start(out=s_An[1][C1:C1 + 1, :, :], in_=ones_src)

            # ---- helpers ----------------------------------------------------
            def w_mult_nm(lhs_fm, w, kdim, fout, name, extra_row=None):
                """nm out: lhsT = fm activation slice [kdim, node_chunk],
                rhs = w [kdim, fout]. Returns (nm0, nm1) scaled by d.
                extra_row: bf16 [1, fout] AP copied into partition 22 of the
                chunk-1 tile (pairs with the An ones-row bias fold)."""
                outs = []
                for c, cn in ((0, C0), (1, C1)):
                    off, _ = _chunk(c)
                    p = ps.tile([cn, GPD, fout], F32, tag="ps")
                    for g in range(GPD):
                        nc.tensor.matmul(
                            p[:, g, :], lhs_fm[0:kdim, g, off:off + cn], w,
                            start=True, stop=True,
                        )
                    rows = cn + (1 if (c == 1 and extra_row is not None) else 0)
                    o = act.tile([rows, GPD, fout], MMDT, tag=f"{name}{c}")
                    dbc = s_d[c][:][:, :, None].broadcast_to((cn, GPD, fout))
                    nc.vector.tensor_mul(o[0:cn, 0:H2, :], p[:, 0:H2, :],
                                         dbc[:, 0:H2, :])
                    nc.vector.tensor_mul(o[0:cn, H2:GPD, :], p[:, H2:GPD, :],
                                         dbc[:, H2:GPD, :])
                    if c == 1 and extra_row is not None:
                        erb = extra_row[:, None, :].broadcast_to((1, GPD, fout))
                        nc.gpsimd.dma_start(out=o[C1:C1 + 1, :, :], in_=erb)
                    outs.append(o)
                return outs

            def an_mult_fm(m_nm, bias, name):
                """fm out [HIDDEN, g, NPG] = relu((An @ M) + bias) per graph.
                lhsT = M_nm chunk [k, HIDDEN], rhs = An_col chunk [k, NPG]."""
                o = act.tile([HIDDEN, GPD, NPG], MMDT, tag=name)
                for g in range(GPD):           # one PSUM bank per graph
                    p = ps.tile([HIDDEN, NPG], F32, tag="ps")
                    for c, cn in ((0, C0), (1, C1)):
                        nc.tensor.matmul(
                            p[:], m_nm[c][0:cn, g, :], s_An[c][0:cn, g, :],
                            start=(c == 0), stop=(c == 1),
                        )
                    nc.scalar.activation(o[:, g, :], p[:], AF.Relu, bias=bias)
                return o

            # ---- encoder ----------------------------------------------------
            m1 = w_mult_nm(s_xT, s_W1, N_FEAT, HIDDEN, "m1")
            z1 = an_mult_fm(m1, s_b1, "z1")                       # [64, g, 150] fm
            m2 = w_mult_nm(z1, s_W2, HIDDEN, HIDDEN, "m2")
            z2 = an_mult_fm(m2, s_b2, "z2")                       # [64, g, 150] fm

            # ---- Z2 transpose -> nm (for pooling contractions) --------------
            z2n = []
            for c, cn in ((0, C0), (1, C1)):
                off, _ = _chunk(c)
                p = pst.tile([cn, GPD, HIDDEN], MMDT, tag="p2")
                for g in range(GPD):
                    nc.tensor.transpose(p[:, g, :], z2[0:HIDDEN, g, off:off + cn],
                                        s_id64)
                o = act.tile([cn, GPD, HIDDEN], MMDT, tag=f"z2n{c}")
                nc.vector.tensor_copy(o[:], p[:])
                z2n.append(o)

            # ---- assignment: S = softmax(An @ (Z2 @ Wa) + ba), nm -----------
            # ba rides the An ones-row: chunk-1 V tile carries ba in row 22.
            v = w_mult_nm(z2, s_Wa, HIDDEN, CLUSTERS, "v",
                          extra_row=wpk[0:1, WP_BA:WP_BA + CLUSTERS])
            s_S = []
            for mc, mn in ((0, C0), (1, C1)):
                moff, _ = _chunk(mc)
                p = ps.tile([mn, GPD, CLUSTERS], F32, tag="ps")
                for g in range(GPD):
                    for c, cn, ck in ((0, C0, C0), (1, C1, C1 + 1)):
                        nc.tensor.matmul(
                            p[:, g, :], s_An[c][0:ck, g, moff:moff + mn],
                            v[c][0:ck, g, :], start=(c == 0), stop=(c == 1),
                        )
                e = act.tile([mn, GPD, CLUSTERS], F32, tag=f"e{mc}")
                nc.scalar.activation(e[:], p[:], AF.Exp)
                ssum = act.tile([mn, GPD], F32, tag=f"ssum{mc}")
                nc.vector.reduce_sum(out=ssum[:], in_=e[:], axis=mybir.AxisListType.X)
                rs = act.tile([mn, GPD], F32, tag=f"rs{mc}")
                nc.vector.reciprocal(rs[:], ssum[:])
                s = act.tile([mn, GPD, CLUSTERS], MMDT, tag=f"s{mc}")
                nc.vector.tensor_mul(s[:], e[:],
                                     rs[:][:, :, None].broadcast_to((mn, GPD, CLUSTERS)))
                s_S.append(s)

            # ---- AS = A @ S (raw adjacency), nm -----------------------------
            s_AS = []
            for mc, mn in ((0, C0), (1, C1)):
                moff, _ = _chunk(mc)
                p = ps.tile([mn, GPD, CLUSTERS], F32, tag="ps")
                for g in range(GPD):
                    for c, cn in ((0, C0), (1, C1)):
                        nc.tensor.matmul(
                            p[:, g, :], s_a[c][0:cn, g, moff:moff + mn],
                            s_S[c][0:cn, g, :], start=(c == 0), stop=(c == 1),
                        )
                o = act.tile([mn, GPD, CLUSTERS], MMDT, tag=f"as{mc}")
                nc.vector.tensor_copy(o[:], p[:])
                s_AS.append(o)

            # ---- Ap = S^T @ AS (nm out) first: its norm chain (DVE) then
            # overlaps the Zp/ZpWp matmuls on PE ------------------------------
            p_ap = ps.tile([CLUSTERS, GPD, CLUSTERS], F32, tag="ps")
            for g in range(GPD):
                for c, cn in ((0, C0), (1, C1)):
                    nc.tensor.matmul(p_ap[:, g, :], s_S[c][0:cn, g, :],
                                     s_AS[c][0:cn, g, :], start=(c == 0), stop=(c == 1))

            p_zp = ps.tile([HIDDEN, GPD, CLUSTERS], F32, tag="ps")
            for g in range(GPD):
                for c, cn in ((0, C0), (1, C1)):
                    nc.tensor.matmul(p_zp[:, g, :], z2n[c][0:cn, g, :],
                                     s_S[c][0:cn, g, :], start=(c == 0), stop=(c == 1))
            s_Zp = act.tile([HIDDEN, GPD, CLUSTERS], MMDT, tag="zp")
            nc.scalar.copy(s_Zp[:], p_zp[:])


            # ---- pooled normalization ---------------------------------------
            degp = act.tile([CLUSTERS, GPD], F32, tag="degp")
            nc.vector.reduce_sum(out=degp[:], in_=p_ap[:], axis=mybir.AxisListType.X)
            nc.vector.tensor_scalar_add(degp[:], degp[:], 1.0)
            dp = emit_rsqrt(degp, CLUSTERS, GPD, iters=1)

            # column degrees of Ap without touching Ap: softmax rows sum to 1,
            # so colsum(Ap) = colsum(AS). Runs as soon as AS lands, fully
            # overlapping the Ap/Zp matmuls.
            p_cs = pst.tile([1, GPD * CLUSTERS], F32, tag="p2")
            onec = s_wpk[:, WP_ONEC:WP_ONEC + 1]
            nc.tensor.matmul(p_cs[:], onec[0:C0, :], s_AS[0][:],
                             start=True, stop=False)
            nc.tensor.matmul(p_cs[:], onec[0:C1, :], s_AS[1][:],
                             start=False, stop=True)
            urow = act.tile([1, GPD * CLUSTERS], F32, tag="urow")
            nc.vector.tensor_scalar_add(urow[:], p_cs[:], 1.0)
            ubc = act.tile([CLUSTERS, GPD * CLUSTERS], F32, tag="ubc")
            nc.gpsimd.partition_broadcast(ubc[:], urow[:])
            dpT2 = emit_rsqrt(ubc, CLUSTERS, GPD * CLUSTERS, iters=1)
            s_dpT = dpT2[:].rearrange("p (g j) -> p g j", g=GPD)

            # Anp = dp_row * (Ap + I) * dp_col, materialized fully (tiny).
            ahp = act.tile([CLUSTERS, GPD, CLUSTERS], F32, tag="ahp")
            id25b = s_fpk[0:CLUSTERS, FP_ID25:FP_ID25 + CLUSTERS][:, None, :] \
                .broadcast_to((CLUSTERS, GPD, CLUSTERS))
            nc.vector.tensor_add(ahp[:], p_ap[:], id25b)
            nc.vector.tensor_mul(ahp[:], ahp[:],
                                 dp[:][:, :, None].broadcast_to((CLUSTERS, GPD, CLUSTERS)))
            anp = act.tile([CLUSTERS, GPD, CLUSTERS], MMDT, tag="anp")
            nc.vector.tensor_mul(anp[:], ahp[:], s_dpT)

            # ---- pooled GCN: H = relu(Anp @ (Zp @ Wp) + bp), fm -------------
            p_zw = ps.tile([CLUSTERS, GPD, HIDDEN], F32, tag="ps")
            for g in range(GPD):
                nc.tensor.matmul(p_zw[:, g, :], s_Zp[:, g, :], s_Wp,
                                 start=True, stop=True)
            s_ZW = act.tile([CLUSTERS, GPD, HIDDEN], MMDT, tag="zw")
            nc.vector.tensor_copy(s_ZW[:], p_zw[:])

            p_h = ps.tile([HIDDEN, GPD, CLUSTERS], F32, tag="ps")
            for g in range(GPD):
                nc.tensor.matmul(p_h[:, g, :], s_ZW[:, g, :], anp[:, g, :],
                                 start=True, stop=True)
            s_H = act.tile([HIDDEN, GPD, CLUSTERS], F32, tag="h")
            nc.scalar.activation(s_H[:], p_h[:], AF.Relu, bias=s_bp)

            # ---- readout + classifier ---------------------------------------
            s_G = act.tile([HIDDEN, GPD], F32, tag="g")
            nc.vector.reduce_sum(out=s_G[:], in_=s_H[:], axis=mybir.AxisListType.X)

            p_l = ps.tile([GPD, NUM_CLASSES], F32, tag="ps")
            nc.tensor.matmul(p_l[:], s_G[:], s_Wc, start=True, stop=True)
            s_out = act.tile([GPD, NUM_CLASSES], F32, tag="logits")
            nc.vector.tensor_add(s_out[:], p_l[:], s_bc)
            nc.sync.dma_start(out=out, in_=s_out[:])

    nc.compile()
    return nc


def make_in_maps(x, a, W1, b1, W2, b2, Wa, ba, Wp, bp, Wc, bc):
    import ml_dtypes
    npmm = np.dtype(ml_dtypes.bfloat16) if MMDT == BF16 else np.dtype(np.float32)

    x = np.ascontiguousarray(np.asarray(x, dtype=np.float32))
    a = np.asarray(a, dtype=np.float32)

    # diagonal 150x150 blocks of the batch adjacency
    ab = a.reshape(B_GRAPHS, NPG, B_GRAPHS, NPG)
    blocks = ab[np.arange(B_GRAPHS), :, np.arange(B_GRAPHS), :]  # [64, 150, 150]
    blocks = blocks.astype(npmm)

    # identities: id0 [128,150] | id1 [22,150] packed side by side
    idp = np.zeros((C0, 2 * NPG), npmm)
    idp[np.arange(C0), np.arange(C0)] = 1.0
    idp[np.arange(C1), NPG + C0 + np.arange(C1)] = 1.0

    wpk = np.zeros((N_FEAT, WP_COLS), npmm)
    wpk[:, WP_W1:WP_W1 + HIDDEN] = np.asarray(W1, np.float32).astype(npmm)
    wpk[0:HIDDEN, WP_W2:WP_W2 + HIDDEN] = np.asarray(W2, np.float32).astype(npmm)
    wpk[0:HIDDEN, WP_WA:WP_WA + CLUSTERS] = np.asarray(Wa, np.float32).astype(npmm)
    wpk[0:HIDDEN, WP_WP:WP_WP + HIDDEN] = np.asarray(Wp, np.float32).astype(npmm)
    wpk[0, WP_BA:WP_BA + CLUSTERS] = np.asarray(ba, np.float32).astype(npmm)
    wpk[:, WP_ID128:WP_ID128 + N_FEAT] = np.eye(N_FEAT, dtype=npmm)
    wpk[0, WP_ONES:WP_ONES + NPG] = 1.0
    wpk[:, WP_ONEC] = 1.0

    fpk = np.zeros((N_FEAT, FP_COLS), np.float32)
    fpk[0:HIDDEN, FP_WC:FP_WC + NUM_CLASSES] = np.asarray(Wc, np.float32)
    fpk[0:HIDDEN, FP_B1] = np.asarray(b1, np.float32)
    fpk[0:HIDDEN, FP_B2] = np.asarray(b2, np.float32)
    fpk[0:HIDDEN, FP_BP] = np.asarray(bp, np.float32)
    fpk[:, FP_BC:FP_BC + NUM_CLASSES] = np.asarray(bc, np.float32)[None, :]
    fpk[0:CLUSTERS, FP_ID25:FP_ID25 + CLUSTERS] = np.eye(CLUSTERS, dtype=np.float32)
    fpk[:, FP_ONE] = 1.0

    common = dict(idp=idp, wpk=wpk, fpk=fpk)

    in_maps = []
    for d in range(DEV):
        xd = x[d * GPD * NPG:(d + 1) * GPD * NPG]          # [1200, 128]
        xT = np.ascontiguousarray(xd.T).reshape(N_FEAT, GPD, NPG).astype(npmm)
        bd = blocks[d * GPD:(d + 1) * GPD]                  # [8, 150, 150]
        bt = np.ascontiguousarray(bd.transpose(1, 0, 2))    # [150, 8, 150]
        in_maps.append(dict(
            xT=xT,
            a0=np.ascontiguousarray(bt[:C0]),
            a1=np.ascontiguousarray(bt[C0:]),
            **common,
        ))
    return in_maps

def kernel(x, a, seg_ids, num_graphs, W1, b1, W2, b2, Wa, ba, Wp, bp, Wc, bc,
           trace=False):
    if "nc" not in _CACHE:
        _CACHE["nc"] = build_nc()
    nc = _CACHE["nc"]
    in_maps = make_in_maps(x, a, W1, b1, W2, b2, Wa, ba, Wp, bp, Wc, bc)
    res = run_bass_kernel_spmd(nc, in_maps, core_ids=list(range(DEV)), trace=trace)
    logits = np.concatenate([r["out"] for r in res.results], axis=0)
    if trace:
        return logits, res
    return logits



# revision 4
# speedup vs baseline: 1.0676x; 1.0676x over previous
"""GCN + DiffPool kernel for Trainium2, data-parallel over graphs across 8 NeuronCores.

Model (per graph, n=150 nodes):
  Z1 = relu(An @ (x @ W1) + b1)          An = D^-1/2 (A+I) D^-1/2
  Z2 = relu(An @ (Z1 @ W2) + b2)
  S  = softmax(An @ (Z2 @ Wa) + ba)      [n, 25]
  Zp = S^T @ Z2 ; Ap = S^T @ (A @ S)
  H  = relu(Anp @ (Zp @ Wp) + bp)        pooled GCN, 25 cluster-nodes
  logits = (sum_rows H) @ Wc + bc

Sharding: 64 graphs -> 8 devices x 8 graphs; each device gets its graphs'
150x150 diagonal blocks of A+I (node chunks c0=[0:128], c1=[128:150] on
partitions) and node rows of x (feature-major). Final [8,10] logits per
device concatenate on host.

All-node-major dataflow. Every activation keeps nodes on partitions, so the
row normalization factor d = rsqrt(deg+1) is a per-partition scale:
    Z1d = d_j * Z1[j,:] = relu(d_j^2 * psum)         (d>0 commutes with relu)
with psum = sum_i Ah[i,j] (d_i M1[i,h]) + dinv_j*b1[h]; the bias rides an
augmented contraction row (Ah row 22 = dinv_j, M1d row 22 = b1), and the
column factor d_j^2 = 1/(deg_j+1) is applied at the PSUM drain. The
propagate matmuls use lhsT = Ah[i, j-slice] directly (A+I is symmetric), so
no An matrix and no transposes are ever materialized. AS = A@S is recovered
from (A+I)@S - S. The pooled stage folds dp past relu into the readout:
  G @ Wc = sum_c' dp_c' relu(psum_h)[:,c'] @ Wc  ->  per-cluster matmul
then a ones-contraction matmul collapses clusters, with bc on an aug row.
"""

import numpy as np

import concourse.bass as bass
import concourse.mybir as mybir
import concourse.tile as tile
from concourse import bacc
from concourse.bass_utils import run_bass_kernel_spmd

F32 = mybir.dt.float32
BF16 = mybir.dt.bfloat16
U32 = mybir.dt.uint32
AF = mybir.ActivationFunctionType
AL = mybir.AluOpType

MMDT = BF16

N_NODES = 9600
N_FEAT = 128
HIDDEN = 64
CLUSTERS = 25
NUM_CLASSES = 10
B_GRAPHS = 64
NPG = 150            # nodes per graph
DEV = 8              # devices
GPD = 8              # graphs per device
C0, C1 = 128, 22     # node partition chunks

# wpk (bf16) column offsets
WP_W1 = 0                       # [128, 64]
WP_W2A = WP_W1 + HIDDEN         # [65, 64]  row 64 = b2
WP_WAA = WP_W2A + HIDDEN        # [65, 25]  row 64 = ba
WP_WP = WP_WAA + CLUSTERS       # [64, 64]
WP_WC = WP_WP + HIDDEN          # [64, 10]
WP_ID = WP_WC + NUM_CLASSES     # [128, 128] identity
WP_B1 = WP_ID + N_FEAT          # row 0: b1 [1, 64]
WP_BP = WP_B1 + HIDDEN          # row 0: bp [1, 64]
WP_COLS = WP_BP + HIDDEN

# fpk (fp32) column offsets
FP_ONES = 0                     # rows 0:26 = 1.0 (ones contraction col)
FP_BC = 1                       # row 0, cols 1:81 = tile(bc, 8)
FP_COLS = FP_BC + GPD * NUM_CLASSES

_CACHE = {}


def _chunk(c):
    return (0, C0) if c == 0 else (C0, C1)


def build_nc():
    nc = bacc.Bacc("TRN2", target_bir_lowering=False, debug=False, num_devices=DEV)

    def din(name, shape, dt=MMDT):
        return nc.dram_tensor(name, shape, dt, kind="ExternalInput").ap()

    xT = din("xT", [N_FEAT, GPD, NPG])
    a0 = din("a0", [C0, GPD, NPG])          # rows 0:128 of A+I blocks
    a1 = din("a1", [C1, GPD, NPG])          # rows 128:150 of A+I blocks
    wpk = din("wpk", [N_FEAT, WP_COLS])
    fpk = din("fpk", [N_FEAT, FP_COLS], F32)
    outd = nc.dram_tensor("out", [GPD * NUM_CLASSES], F32, kind="ExternalOutput").ap()

    with tile.TileContext(nc) as tc:
        with (
            tc.tile_pool(name="cst", bufs=1) as cst,
            tc.tile_pool(name="act", bufs=1) as act,
            tc.tile_pool(name="ps", bufs=3, space="PSUM") as ps,
            tc.tile_pool(name="psu", bufs=2, space="PSUM") as psu,
            tc.tile_pool(name="pst", bufs=1, space="PSUM") as pst,
            tc.tile_pool(name="pw", bufs=1, space="PSUM") as pwp,
            tc.tile_pool(name="dram", bufs=1, space="DRAM") as dram,
        ):
            H2 = GPD // 2

            # ---- PE warmup: keep HAM busy while DMAs land ------------------
            warm = cst.tile([C0, 256], MMDT, tag="warm")
            nc.vector.memset(warm[:], 1)
            pwt = pwp.tile([C0, 256], F32, tag="pw")
            for _ in range(14):
                nc.tensor.matmul(pwt[:], warm[:, 0:C0], warm[:],
                                 start=True, stop=True)

            # ---- input DMAs ------------------------------------------------
            s_a0 = cst.tile([C0, GPD, NPG], MMDT, tag="a0")
            nc.sync.dma_start(out=s_a0[:, 0:H2, :], in_=a0[:, 0:H2, :])
            nc.scalar.dma_start(out=s_a0[:, H2:GPD, :], in_=a0[:, H2:GPD, :])
            s_a1 = cst.tile([C1 + 1, GPD, NPG], MMDT, tag="a1")
            nc.sync.dma_start(out=s_a1[0:C1, :, :], in_=a1)
            s_xT = cst.tile([N_FEAT, GPD, NPG], MMDT, tag="xT")
            nc.gpsimd.dma_start(out=s_xT[:], in_=xT)
            s_wpk = cst.tile([N_FEAT, WP_COLS], MMDT, tag="wpk")
            nc.gpsimd.dma_start(out=s_wpk[:], in_=wpk)
            s_fpk = cst.tile([N_FEAT, FP_COLS], F32, tag="fpk")
            nc.scalar.dma_start(out=s_fpk[:], in_=fpk)

            s_a = (s_a0, s_a1)
            s_W1 = s_wpk[:, WP_W1:WP_W1 + HIDDEN]
            s_W2a = s_wpk[0:HIDDEN + 1, WP_W2A:WP_W2A + HIDDEN]
            s_Waa = s_wpk[0:HIDDEN + 1, WP_WAA:WP_WAA + CLUSTERS]
            s_Wp = s_wpk[0:HIDDEN, WP_WP:WP_WP + HIDDEN]
            s_Wc = s_wpk[0:HIDDEN, WP_WC:WP_WC + NUM_CLASSES]
            s_id = s_wpk[:, WP_ID:WP_ID + N_FEAT]
            s_ones26 = s_fpk[0:CLUSTERS + 1, FP_ONES:FP_ONES + 1]

            # ---- degrees: rowsum(A+I) = deg+1 on partitions ----------------
            degc = act.tile([C0, 2 * GPD], F32, tag="degc")
            nc.vector.memset(degc[0:C0, GPD:2 * GPD], 1.0)
            nc.vector.reduce_sum(out=degc[:, 0:H2], in_=s_a0[:, 0:H2, :],
                                 axis=mybir.AxisListType.X)
            nc.vector.reduce_sum(out=degc[:, H2:GPD], in_=s_a0[:, H2:GPD, :],
                                 axis=mybir.AxisListType.X)
            nc.vector.reduce_sum(out=degc[0:C1, GPD:2 * GPD], in_=s_a1[0:C1, :, :],
                                 axis=mybir.AxisListType.X)

            qk1 = act.tile([C0, 1], U32, tag="qk1")
            nc.vector.memset(qk1[:], 1)
            qkm = act.tile([C0, 1], U32, tag="qkm")
            nc.vector.memset(qkm[:], 0x5F3759DF)

            def emit_rsqrt(x, rows, cols, iters=2):
                s = act.tile([rows, cols], F32, tag=f"rs_{id(x)}")
                w = act.tile([rows, cols], F32, tag=f"rw_{id(x)}")
                nc.vector.tensor_tensor(s[:].bitcast(U32), x[:].bitcast(U32),
                                        qk1[0:rows, :].broadcast_to((rows, cols)),
                                        AL.logical_shift_right)
                nc.vector.tensor_tensor(s[:].bitcast(U32),
                                        qkm[0:rows, :].broadcast_to((rows, cols)),
                                        s[:].bitcast(U32), AL.subtract)
                for _ in range(iters):
                    nc.vector.tensor_mul(w[:], s[:], s[:])
                    nc.vector.tensor_mul(w[:], w[:], x[:])
                    nc.vector.tensor_scalar(w[:], w[:], -0.5, 1.5, AL.mult, AL.add)
                    nc.vector.tensor_mul(s[:], s[:], w[:])
                return s

            dcomb = emit_rsqrt(degc, C0, 2 * GPD)          # d = rsqrt(deg+1)
            d2comb = act.tile([C0, 2 * GPD], F32, tag="d2c")
            nc.vector.reciprocal(d2comb[:], degc[:])       # d^2 = 1/(deg+1)
            dinvc = act.tile([C0, 2 * GPD], F32, tag="dic")
            nc.vector.tensor_mul(dinvc[:], dcomb[:], degc[:])   # 1/d = sqrt(deg+1)
            dinvb = act.tile([C0, 2 * GPD], MMDT, tag="dib")
            nc.vector.tensor_copy(dinvb[:], dinvc[:])

            s_d = [dcomb[:, 0:GPD], dcomb[0:C1, GPD:2 * GPD]]
            s_d2 = [d2comb[:, 0:GPD], d2comb[0:C1, GPD:2 * GPD]]
            s_dinvb = [dinvb[:, 0:GPD], dinvb[0:C1, GPD:2 * GPD]]

            # ---- dinv as a free-dim row via transpose + DRAM hop -----------
            p_dt = pst.tile([GPD * 2, 160], MMDT, tag="ptr")
            nc.tensor.transpose(p_dt[0:GPD, 0:C0], s_dinvb[0][:], s_id)
            nc.tensor.transpose(p_dt[0:GPD, C0:NPG], s_dinvb[1][:],
                                s_id[0:C1, 0:C1])
            dTrow = act.tile([GPD, NPG], MMDT, tag="dTrow")
            nc.vector.tensor_copy(dTrow[:], p_dt[0:GPD, 0:NPG])
            dTd = dram.tile([GPD * NPG], MMDT, tag="dTd")
            nc.sync.dma_start(out=dTd[:].rearrange("(g j) -> g j", g=GPD),
                              in_=dTrow[:])
            dinv_row = dTd[:].rearrange("(g j) -> g j", g=GPD)[None, :, :]
            # aug row 22 of the chunk-1 adjacency: dinv_j
            nc.sync.dma_start(out=s_a1[C1:C1 + 1, :, :], in_=dinv_row)

            # ---- M1 = (X @ W1) * d_row, node-major; aug row = b1 -----------
            m1 = []
            for c, cn in ((0, C0), (1, C1)):
                off, _ = _chunk(c)
                p = ps.tile([C0, GPD, HIDDEN], F32, tag="ps")
                for g in range(GPD):
                    nc.tensor.matmul(p[0:cn, g, :],
                                     s_xT[:, g, off:off + cn], s_W1,
                                     start=True, stop=True)
                rows = cn + (1 if c == 1 else 0)
                o = act.tile([rows, GPD, HIDDEN], MMDT, tag=f"m1_{c}")
                dbc = s_d[c][:][:, :, None].broadcast_to((cn, GPD, HIDDEN))
                nc.vector.tensor_mul(o[0:cn, :, :], p[0:cn, :, :], dbc)
                if c == 1:
                    b1b = wpk[0:1, WP_B1:WP_B1 + HIDDEN][:, None, :] \
                        .broadcast_to((1, GPD, HIDDEN))
                    nc.gpsimd.dma_start(out=o[C1:C1 + 1, :, :], in_=b1b)
                m1.append(o)

            def prop_nm(rhs_tiles, d2s, name, fout=HIDDEN):
                """Z[j,h] = relu(d_j^2 * sum_i Ah_aug[i,j] rhs_aug[i,h]).
                rhs tiles: (c0 [128,g,fout], c1 [23,g,fout] w/ aug row)."""
                outs = []
                for jc, jn in ((0, C0), (1, C1)):
                    joff, _ = _chunk(jc)
                    p = ps.tile([C0, GPD, fout], F32, tag="ps")
                    for g in range(GPD):
                        nc.tensor.matmul(p[0:jn, g, :],
                                         s_a0[:, g, joff:joff + jn],
                                         rhs_tiles[0][0:C0, g, :],
                                         start=True, stop=False)
                        nc.tensor.matmul(p[0:jn, g, :],
                                         s_a1[0:C1 + 1, g, joff:joff + jn],
                                         rhs_tiles[1][0:C1 + 1, g, :],
                                         start=False, stop=True)
                    o = act.tile([jn, GPD, fout], MMDT, tag=f"{name}{jc}")
                    d2bc = d2s[jc][:][:, :, None].broadcast_to((jn, GPD, fout))
                    nc.vector.scalar_tensor_tensor(
                        o[:], p[0:jn, :, :], 0.0, d2bc, AL.max, AL.mult)
                    outs.append(o)
                return outs

            # ---- layer 1: Z1d = d * relu(An@M1 + b1) -----------------------
            z1d = prop_nm(m1, s_d2, "z1d")

            # ---- U = raw(An @ Z1), feature-major; aug row 64 = dinv --------
            def an_prop_fm(lhs_tiles, name):
                o = act.tile([HIDDEN + 1, GPD, NPG], MMDT, tag=name)
                nc.scalar.dma_start(out=o[HIDDEN:HIDDEN + 1, :, :], in_=dinv_row)
                for h in range(0, GPD, 2):
                    p = psu.tile([HIDDEN, 2, 256], F32, tag="psu")
                    for gg in range(2):
                        g = h + gg
                        for c, cn in ((0, C0), (1, C1)):
                            off, _ = _chunk(c)
                            nc.tensor.matmul(p[:, gg, 0:NPG],
                                             lhs_tiles[c][0:cn, g, :],
                                             s_a[c][0:cn, g, :],
                                             start=(c == 0), stop=(c == 1))
                    nc.scalar.copy(o[0:HIDDEN, h:h + 2, :], p[:, :, 0:NPG])
                return o

            u = an_prop_fm(z1d, "u")

            # ---- layer 2: Z2d = d * relu((U@W2)*d + b2) --------------------
            def w_stage_nm(lhs_fm, w_aug, d2s, name, fout=HIDDEN, relu=True):
                """out[j,:] = drain(d_j^2 * sum_h lhs_aug[h,j] w_aug[h,:])."""
                outs = []
                for jc, jn in ((0, C0), (1, C1)):
                    joff, _ = _chunk(jc)
                    p = ps.tile([C0, GPD, fout], F32, tag="ps")
                    for g in range(GPD):
                        nc.tensor.matmul(p[0:jn, g, :],
                                         lhs_fm[0:HIDDEN + 1, g, joff:joff + jn],
                                         w_aug, start=True, stop=True)
                    outs.append(p)
                return outs

            p2 = w_stage_nm(u, s_W2a, s_d2, "p2")
            z2d = []
            for jc, jn in ((0, C0), (1, C1)):
                o = act.tile([jn, GPD, HIDDEN], MMDT, tag=f"z2d{jc}")
                d2bc = s_d2[jc][:][:, :, None].broadcast_to((jn, GPD, HIDDEN))
                nc.vector.scalar_tensor_tensor(
                    o[:], p2[jc][0:jn, :, :], 0.0, d2bc, AL.max, AL.mult)
                z2d.append(o)

            # ---- T = raw(An @ Z2), then P = T@Wa, softmax ------------------
            t = an_prop_fm(z2d, "t")
            pp = w_stage_nm(t, s_Waa, None, "pp", fout=CLUSTERS)

            s_S, s_Si = [], []
            for jc, jn in ((0, C0), (1, C1)):
                pm = act.tile([jn, GPD, CLUSTERS], F32, tag=f"pm{jc}")
                dbc = s_d[jc][:][:, :, None].broadcast_to((jn, GPD, CLUSTERS))
                nc.vector.tensor_mul(pm[:], pp[jc][0:jn, :, :], dbc)
                e = act.tile([jn, GPD, CLUSTERS], F32, tag=f"e{jc}")
                nc.scalar.activation(e[:], pm[:], AF.Exp)
                ssum = act.tile([jn, GPD], F32, tag=f"ssum{jc}")
                nc.vector.reduce_sum(out=ssum[:], in_=e[:],
                                     axis=mybir.AxisListType.X)
                rs = act.tile([jn, GPD], F32, tag=f"rsx{jc}")
                nc.vector.reciprocal(rs[:], ssum[:])
                s = act.tile([jn, GPD, CLUSTERS], MMDT, tag=f"s{jc}")
                nc.vector.tensor_mul(
                    s[:], e[:], rs[:][:, :, None].broadcast_to((jn, GPD, CLUSTERS)))
                s_S.append(s)
                si = act.tile([jn, GPD, CLUSTERS], MMDT, tag=f"si{jc}")
                dib = s_dinvb[jc][:][:, :, None].broadcast_to((jn, GPD, CLUSTERS))
                nc.vector.tensor_mul(si[:], s[:], dib)
                s_Si.append(si)

            # ---- AS = (A+I)@S - S, node-major ------------------------------
            s_AS = []
            for jc, jn in ((0, C0), (1, C1)):
                joff, _ = _chunk(jc)
                p = ps.tile([C0, GPD, CLUSTERS], F32, tag="ps")
                for g in range(GPD):
                    for c, cn in ((0, C0), (1, C1)):
                        nc.tensor.matmul(p[0:jn, g, :],
                                         s_a[c][0:cn, g, joff:joff + jn],
                                         s_S[c][0:cn, g, :],
                                         start=(c == 0), stop=(c == 1))
                o = act.tile([jn, GPD, CLUSTERS], MMDT, tag=f"as{jc}")
                nc.vector.tensor_sub(o[:], p[0:jn, :, :], s_S[jc][:])
                s_AS.append(o)

            # ---- Ap = S^T @ AS ; Zp^T = Z2^T @ S ---------------------------
            p_ap = ps.tile([CLUSTERS, GPD, CLUSTERS], F32, tag="ps")
            for g in range(GPD):
                for c, cn in ((0, C0), (1, C1)):
                    nc.tensor.matmul(p_ap[:, g, :], s_S[c][0:cn, g, :],
                                     s_AS[c][0:cn, g, :],
                                     start=(c == 0), stop=(c == 1))
            p_zp = ps.tile([HIDDEN, GPD, CLUSTERS], F32, tag="ps")
            for g in range(GPD):
                for c, cn in ((0, C0), (1, C1)):
                    nc.tensor.matmul(p_zp[:, g, :], z2d[c][0:cn, g, :],
                                     s_Si[c][0:cn, g, :],
                                     start=(c == 0), stop=(c == 1))
            s_Zp = act.tile([HIDDEN, GPD, CLUSTERS], MMDT, tag="zp")
            nc.scalar.copy(s_Zp[:], p_zp[:])

            # ---- pooled normalization --------------------------------------
            degp = act.tile([CLUSTERS, GPD], F32, tag="degp")
            nc.vector.reduce_sum(out=degp[:], in_=p_ap[:],
                                 axis=mybir.AxisListType.X)
            nc.vector.tensor_scalar_add(degp[:], degp[:], 1.0)
            dp = emit_rsqrt(degp, CLUSTERS, GPD, iters=1)
            dinvp = act.tile([CLUSTERS, GPD], MMDT, tag="dinvp")
            dinvpf = act.tile([CLUSTERS, GPD], F32, tag="dinvpf")
            nc.vector.tensor_mul(dinvpf[:], dp[:], degp[:])
            nc.vector.tensor_copy(dinvp[:], dinvpf[:])

            p_dp = pst.tile([GPD * 2, 160], MMDT, tag="ptr")
            nc.tensor.transpose(p_dp[0:GPD, 0:CLUSTERS], dinvp[:],
                                s_id[0:CLUSTERS, 0:CLUSTERS])
            dprow = act.tile([GPD, CLUSTERS], MMDT, tag="dprow")
            nc.vector.tensor_copy(dprow[:], p_dp[0:GPD, 0:CLUSTERS])
            dpd = dram.tile([GPD * CLUSTERS], MMDT, tag="dpd")
            nc.sync.dma_start(out=dpd[:].rearrange("(g c) -> g c", g=GPD),
                              in_=dprow[:])

            # Ahp = Ap + I, with aug row 25 = dinvp_col
            ahp = act.tile([CLUSTERS + 1, GPD, CLUSTERS], MMDT, tag="ahp")
            nc.sync.dma_start(
                out=ahp[CLUSTERS:CLUSTERS + 1, :, :],
                in_=dpd[:].rearrange("(g c) -> g c", g=GPD)[None, :, :])
            id25b = s_id[0:CLUSTERS, 0:CLUSTERS][:, None, :] \
                .broadcast_to((CLUSTERS, GPD, CLUSTERS))
            nc.vector.tensor_add(ahp[0:CLUSTERS, :, :], p_ap[:], id25b)

            # ---- pooled GCN ------------------------------------------------
            p_zw = ps.tile([CLUSTERS, GPD, HIDDEN], F32, tag="ps")
            for g in range(GPD):
                nc.tensor.matmul(p_zw[:, g, :], s_Zp[:, g, :], s_Wp,
                                 start=True, stop=True)
            zwd = act.tile([CLUSTERS + 1, GPD, HIDDEN], MMDT, tag="zwd")
            bpb = wpk[0:1, WP_BP:WP_BP + HIDDEN][:, None, :] \
                .broadcast_to((1, GPD, HIDDEN))
            nc.gpsimd.dma_start(out=zwd[CLUSTERS:CLUSTERS + 1, :, :], in_=bpb)
            dpbc = dp[:][:, :, None].broadcast_to((CLUSTERS, GPD, HIDDEN))
            nc.vector.tensor_mul(zwd[0:CLUSTERS, :, :], p_zw[:], dpbc)

            p_h = ps.tile([HIDDEN, GPD, CLUSTERS], F32, tag="ps")
            for g in range(GPD):
                nc.tensor.matmul(p_h[:, g, :], zwd[0:CLUSTERS + 1, g, :],
                                 ahp[0:CLUSTERS + 1, g, :],
                                 start=True, stop=True)
            y = act.tile([HIDDEN, GPD, CLUSTERS], MMDT, tag="y")
            nc.scalar.activation(y[:], p_h[:], AF.Relu)

            # ---- readout: logits = sum_c' dp_c' (Y^T Wc)[c',:] + bc --------
            p_l = ps.tile([CLUSTERS, GPD, NUM_CLASSES], F32, tag="ps")
            for g in range(GPD):
                nc.tensor.matmul(p_l[:, g, :], y[:, g, :], s_Wc,
                                 start=True, stop=True)
            ldp = act.tile([CLUSTERS + 1, GPD, NUM_CLASSES], F32, tag="ldp")
            nc.sync.dma_start(
                out=ldp[CLUSTERS:CLUSTERS + 1, :, :],
                in_=fpk[0:1, FP_BC:FP_BC + GPD * NUM_CLASSES]
                .rearrange("one (g c) -> one g c", g=GPD))
            dpb2 = dp[:][:, :, None].broadcast_to((CLUSTERS, GPD, NUM_CLASSES))
            nc.vector.tensor_mul(ldp[0:CLUSTERS, :, :], p_l[:], dpb2)

            p_f = pst.tile([GPD * NUM_CLASSES, 1], F32, tag="pf")
            nc.tensor.matmul(
                p_f[:],
                ldp[0:CLUSTERS + 1, :, :].rearrange("p g c -> p (g c)"),
                s_ones26, start=True, stop=True)
            s_out = act.tile([GPD * NUM_CLASSES, 1], F32, tag="logits")
            nc.vector.tensor_copy(s_out[:], p_f[:])
            nc.sync.dma_start(
                out=outd[:].rearrange("(p one) -> p one", one=1), in_=s_out[:])

    nc.compile()
    return nc


def make_in_maps(x, a, W1, b1, W2, b2, Wa, ba, Wp, bp, Wc, bc):
    import ml_dtypes
    npmm = np.dtype(ml_dtypes.bfloat16)

    x = np.ascontiguousarray(np.asarray(x, dtype=np.float32))
    a = np.asarray(a, dtype=np.float32)

    ab = a.reshape(B_GRAPHS, NPG, B_GRAPHS, NPG)
    blocks = ab[np.arange(B_GRAPHS), :, np.arange(B_GRAPHS), :].copy()
    blocks[:, np.arange(NPG), np.arange(NPG)] += 1.0    # A + I
    blocks = blocks.astype(npmm)

    wpk = np.zeros((N_FEAT, WP_COLS), npmm)
    wpk[:, WP_W1:WP_W1 + HIDDEN] = np.asarray(W1, np.float32).astype(npmm)
    wpk[0:HIDDEN, WP_W2A:WP_W2A + HIDDEN] = np.asarray(W2, np.float32).astype(npmm)
    wpk[HIDDEN, WP_W2A:WP_W2A + HIDDEN] = np.asarray(b2, np.float32).astype(npmm)
    wpk[0:HIDDEN, WP_WAA:WP_WAA + CLUSTERS] = np.asarray(Wa, np.float32).astype(npmm)
    wpk[HIDDEN, WP_WAA:WP_WAA + CLUSTERS] = np.asarray(ba, np.float32).astype(npmm)
    wpk[0:HIDDEN, WP_WP:WP_WP + HIDDEN] = np.asarray(Wp, np.float32).astype(npmm)
    wpk[0:HIDDEN, WP_WC:WP_WC + NUM_CLASSES] = np.asarray(Wc, np.float32).astype(npmm)
    wpk[:, WP_ID:WP_ID + N_FEAT] = np.eye(N_FEAT, dtype=npmm)
    wpk[0, WP_B1:WP_B1 + HIDDEN] = np.asarray(b1, np.float32).astype(npmm)
    wpk[0, WP_BP:WP_BP + HIDDEN] = np.asarray(bp, np.float32).astype(npmm)

    fpk = np.zeros((N_FEAT, FP_COLS), np.float32)
    fpk[0:CLUSTERS + 1, FP_ONES] = 1.0
    fpk[0, FP_BC:FP_BC + GPD * NUM_CLASSES] = np.tile(
        np.asarray(bc, np.float32), GPD)

    common = dict(wpk=wpk, fpk=fpk)

    in_maps = []
    for d in range(DEV):
        xd = x[d * GPD * NPG:(d + 1) * GPD * NPG]
        xTl = np.ascontiguousarray(xd.T).reshape(N_FEAT, GPD, NPG).astype(npmm)
        bd = blocks[d * GPD:(d + 1) * GPD]
        bt = np.ascontiguousarray(bd.transpose(1, 0, 2))
        in_maps.append(dict(
            xT=xTl,
            a0=np.ascontiguousarray(bt[:C0]),
            a1=np.ascontiguousarray(bt[C0:]),
            **common,
        ))
    return in_maps


def kernel(x, a, seg_ids, num_graphs, W1, b1, W2, b2, Wa, ba, Wp, bp, Wc, bc,
           trace=False):
    if "nc" not in _CACHE:
        _CACHE["nc"] = build_nc()
    nc = _CACHE["nc"]
    in_maps = make_in_maps(x, a, W1, b1, W2, b2, Wa, ba, Wp, bp, Wc, bc)
    res = run_bass_kernel_spmd(nc, in_maps, core_ids=list(range(DEV)), trace=trace)
    logits = np.concatenate(
        [r["out"].reshape(GPD, NUM_CLASSES) for r in res.results], axis=0)
    if trace:
        return logits, res
    return logits
